# revision 5
# baseline (speedup 1.0000x reference)
"""Trainium2 Bass kernel v2 for nn_MhcModule_41798621724799.

Design vs v1 baseline (81us graded / 90.9us sim):
  - block-diag mix: 32 tokens x 4 streams packed on partitions (p=tt*4+j);
    PE partition-contraction does the stream-mix -> 4x fewer PE columns.
    x_g in that layout comes from one cheap SBUF->SBUF DMA (the layout's
    flat addresses are consecutive 1024-elem runs). The block-diag lhsT is
    built by 4 tiny scatter-DMAs (M_perm) + a constant-mask multiply on the
    otherwise-idle Pool engine.
  - DMA work spread across all 3 queues (SP/ACT/Pool hwdge+swdge).
  - f32->bf16 cast on Pool; w-matmul in bf16 via PE transposes.
  - sinkhorn (5 iters) on Pool via divide; per-group stores right after
    each mix-psum evacuation (short drain tail).
"""

import sys

sys.path.insert(0, "/opt/trn_rl_repo")

import functools
from contextlib import ExitStack

import numpy as np

import concourse.bacc as bacc
import concourse.bass as bass
import concourse.tile as tile
from concourse import mybir
import concourse.hw_specs as hw_specs
from concourse.bass_utils import run_bass_kernel_spmd
from concourse.masks import make_identity

NS = 4
EPS = 1e-6
B, S, D = 1, 4096, 4096
N_CORES = 8
S_SHARD = S // N_CORES  # 512
P = 128
NT = S_SHARD // P  # 4 token tiles per core
NK = 2 * NS + NS * NS  # 24
SINK_ITERS = 4

SINK_POOL = True
MIX_EVAC_ENG = ["v", "v", "v", "a"]
EVACT_ENG = ["v", "v", "v", "v"]

f32 = mybir.dt.float32
f32r = mybir.dt.float32r
bf16 = mybir.dt.bfloat16
AF = mybir.ActivationFunctionType
OP = mybir.AluOpType
AX = mybir.AxisListType


@functools.cache
def _patched_act_tables(module_arch):
    tabs = hw_specs.get_activation_tables(module_arch)
    combined = "natural_log_exp_and_others"
    if combined in tabs:
        special = {AF.Exp, AF.Ln, AF.Square}
        tabs = {
            k: (set(v) if k == combined else set(v) - special)
            for k, v in tabs.items()
        }
    return tabs


bacc.get_activation_tables = _patched_act_tables


def _ap(t, offset, dims):
    return bass.AP(tensor=t, offset=offset, ap=dims)


def _sinkhorn(nc, sinkpool, h, nt, iters):
    """In-place sinkhorn on h [P, nt, NS(j), NS(i)]."""
    if SINK_POOL:
        # reduces+recips on DVE; the big multiplies on Pool
        for _ in range(iters):
            rsum = sinkpool.tile([P, nt, NS], f32, tag="rsum", bufs=2)
            nc.vector.tensor_reduce(out=rsum, in_=h[:], axis=AX.X, op=OP.add)
            rrec = sinkpool.tile([P, nt, NS], f32, tag="rrec", bufs=2)
            nc.vector.reciprocal_approx_fast(out=rrec, in_=rsum)
            ra = rrec[:]
            rb = _ap(ra.tensor, ra.offset, [ra.ap[0], ra.ap[1], ra.ap[2], [0, NS]])
            nc.gpsimd.tensor_tensor(out=h[:], in0=h[:], in1=rb, op=OP.mult)
            csum = sinkpool.tile([P, nt, NS], f32, tag="csum", bufs=2)
            nc.vector.tensor_reduce(
                out=csum, in_=h[:].rearrange("p t j i -> p t i j"), axis=AX.X, op=OP.add
            )
            crec = sinkpool.tile([P, nt, NS], f32, tag="crec", bufs=2)
            nc.vector.reciprocal_approx_fast(out=crec, in_=csum)
            ca = crec[:]
            cb = _ap(ca.tensor, ca.offset, [ca.ap[0], ca.ap[1], [0, NS], ca.ap[2]])
            nc.gpsimd.tensor_tensor(out=h[:], in0=h[:], in1=cb, op=OP.mult)
    else:
        for _ in range(iters):
            rsum = sinkpool.tile([P, nt, NS], f32, tag="rsum", bufs=2)
            nc.vector.tensor_reduce(out=rsum, in_=h[:], axis=AX.X, op=OP.add)
            rrec = sinkpool.tile([P, nt, NS], f32, tag="rrec", bufs=2)
            nc.vector.reciprocal_approx_fast(out=rrec, in_=rsum)
            ra = rrec[:]
            rb = _ap(ra.tensor, ra.offset, [ra.ap[0], ra.ap[1], ra.ap[2], [0, NS]])
            nc.vector.tensor_mul(out=h[:], in0=h[:], in1=rb)
            csum = sinkpool.tile([P, nt, NS], f32, tag="csum", bufs=2)
            nc.vector.tensor_reduce(
                out=csum, in_=h[:].rearrange("p t j i -> p t i j"), axis=AX.X, op=OP.add
            )
            crec = sinkpool.tile([P, nt, NS], f32, tag="crec", bufs=2)
            nc.vector.reciprocal_approx_fast(out=crec, in_=csum)
            ca = crec[:]
            cb = _ap(ca.tensor, ca.offset, [ca.ap[0], ca.ap[1], [0, NS], ca.ap[2]])
            nc.vector.tensor_mul(out=h[:], in0=h[:], in1=cb)


def _mhc_body(ctx, tc, x, phi, alpha, beta, gamma, out, reps=1):
    nc = tc.nc

    consts = ctx.enter_context(tc.tile_pool(name="consts", bufs=1))
    xpool = ctx.enter_context(tc.tile_pool(name="xpool", bufs=2))
    xbfpool = ctx.enter_context(tc.tile_pool(name="xbfpool", bufs=2))
    xtpool = ctx.enter_context(tc.tile_pool(name="xtpool", bufs=2))
    xgpool = ctx.enter_context(tc.tile_pool(name="xgpool", bufs=2))
    outpool = ctx.enter_context(tc.tile_pool(name="outpool", bufs=2))
    smalls = ctx.enter_context(tc.tile_pool(name="smalls", bufs=2 * NT))
    hpool = ctx.enter_context(tc.tile_pool(name="hpool", bufs=2))
    mpool = ctx.enter_context(tc.tile_pool(name="mpool", bufs=2))
    wsbpool = ctx.enter_context(tc.tile_pool(name="wsbpool", bufs=1))
    sink = ctx.enter_context(tc.tile_pool(name="sink", bufs=4))

    psum_t = ctx.enter_context(tc.tile_pool(name="psum_t", bufs=2, space="PSUM"))
    psum_w = ctx.enter_context(tc.tile_pool(name="psum_w", bufs=1, space="PSUM"))
    psum_wt = ctx.enter_context(tc.tile_pool(name="psum_wt", bufs=1, space="PSUM"))

    # ---------------- constants ----------------
    ident_bf = consts.tile([P, P], bf16)
    make_identity(nc, ident_bf)
    ident_f = consts.tile([P, P], f32)
    nc.vector.tensor_copy(out=ident_f, in_=ident_bf)

    # E mask [P, 4(i), 32(tt')]: E[p, i, tt'] = 1 iff p//4 == tt'
    E = consts.tile([P, NS, 32], bf16)
    nc.gpsimd.memset(E[:], 1.0)
    nc.gpsimd.affine_select(
        out=E[:], in_=E[:], compare_op=OP.is_ge, fill=0.0,
        base=0, pattern=[[0, NS], [-4, 32]], channel_multiplier=1,
    )
    nc.gpsimd.affine_select(
        out=E[:], in_=E[:], compare_op=OP.is_ge, fill=0.0,
        base=3, pattern=[[0, NS], [4, 32]], channel_multiplier=-1,
    )

    # phi96 load: phi rearranged [(k r), c] with r=4 (4KB/partition)
    phi96 = consts.tile([96, 1024], f32)
    nc.scalar.dma_start(out=phi96, in_=phi.rearrange("k (r c) -> (k r) c", r=4))
    gamma_q = consts.tile([P, 32], f32)
    nc.sync.dma_start(
        out=gamma_q, in_=_ap(gamma.tensor, gamma.offset, [[1, P], [P, 32]])
    )
    alpha_bc = consts.tile([P, 3], f32)
    nc.gpsimd.dma_start(
        out=alpha_bc, in_=_ap(alpha.tensor, alpha.offset, [[0, P], alpha.ap[0]])
    )
    beta_bc = consts.tile([P, NK], f32)
    nc.gpsimd.dma_start(
        out=beta_bc, in_=_ap(beta.tensor, beta.offset, [[0, P], beta.ap[0]])
    )
    eps_t = consts.tile([P, 1], f32)
    nc.vector.memset(eps_t, EPS)

    # phiT[p, b, k] = phi[k, b*128+p] * gamma[b*128+p] * alpha(k), bf16
    phiT_f = consts.tile([P, 32, NK], f32)
    with tc.tile_pool(name="psum_phi", bufs=1, space="PSUM") as psum_phi:
        for cb in range(8):
            pt = psum_phi.tile([P, 96], f32, tag="psum_phi")
            nc.tensor.transpose(pt, phi96[:, cb * P : (cb + 1) * P], ident_f[:96, :96])
            # phiT_f[p, r*8+cb, k] = pt[p, k*4+r] * gamma_q[p, r*8+cb]
            pf = phiT_f[:]
            dst = _ap(pf.tensor, pf.offset + cb * NK, [pf.ap[0], [8 * NK, 4], [1, NK]])
            pa = pt[:]
            src0 = _ap(pa.tensor, pa.offset, [pa.ap[0], [1, 4], [4, NK]])
            gq = gamma_q[:]
            src1 = _ap(gq.tensor, gq.offset + cb, [gq.ap[0], [8, 4], [0, NK]])
            nc.vector.tensor_tensor(out=dst, in0=src0, in1=src1, op=OP.mult)
    phiT = consts.tile([P, 32, NK], bf16)
    for kr, (k0, k1) in enumerate([(0, NS), (NS, 2 * NS), (2 * NS, NK)]):
        nc.vector.tensor_scalar_mul(
            out=phiT[:, :, k0:k1],
            in0=phiT_f[:, :, k0:k1],
            scalar1=alpha_bc[:, kr : kr + 1],
        )

    # zero w staging buffers once (wtp transpose reads rows 24..127)
    w_sb = []
    for _ in range(2):
        wt = wsbpool.tile([P, P], bf16, tag=f"w_sb{_}")
        nc.gpsimd.memset(wt[:], 0.0)
        w_sb.append(wt)

    psum_m = ctx.enter_context(tc.tile_pool(name="psum_m", bufs=2, space="PSUM"))

    for _rep in range(reps):
        _mhc_rep(tc, x, out, xpool, xbfpool, xtpool, xgpool, outpool, smalls,
                 hpool, mpool, w_sb, sink, psum_t, psum_w, psum_wt, psum_m,
                 ident_bf, E, phiT, beta_bc, eps_t)


def _mhc_rep(tc, x, out, xpool, xbfpool, xtpool, xgpool, outpool, smalls,
             hpool, mpool, w_sb, sink, psum_t, psum_w, psum_wt, psum_m,
             ident_bf, E, phiT, beta_bc, eps_t):
    nc = tc.nc
    h01_tiles = []
    xg_tiles = []
    xbf_tiles = []
    rs_tiles = []

    # one load per queue up-front (x0,x3 on idle SP; x1 on ACT; x2 on Pool)
    load_eng = [nc.sync, nc.scalar, nc.gpsimd, nc.sync]

    def tile_front(t, x_t):
        # cast to bf16 on Pool, in 4 chunks (pipelines with the transposes)
        xbf = xbfpool.tile([P, D], bf16, tag="xbf")
        for q in range(4):
            nc.gpsimd.tensor_copy(
                out=xbf[:, q * 1024 : (q + 1) * 1024],
                in_=x_t[:, q * 1024 : (q + 1) * 1024],
            )

        # RMS on ACT: ssq via Square+accum (dummy main out via 0-stride)
        sqs = smalls.tile([P, 1], f32, tag="sqs", bufs=2)
        ssq = smalls.tile([P, 1], f32, tag="ssq", bufs=2)
        sq_ap = sqs[:]
        nc.scalar.activation(
            out=_ap(sq_ap.tensor, sq_ap.offset, [sq_ap.ap[0], [0, D]]),
            in_=x_t[:], func=AF.Square, accum_out=ssq,
        )
        lnm = smalls.tile([P, 1], f32, tag="lnm", bufs=2)
        nc.scalar.activation(out=lnm, in_=ssq, func=AF.Ln, scale=1.0 / D, bias=eps_t[:])
        rs = smalls.tile([P, 1], f32, tag="rs")
        nc.scalar.activation(out=rs, in_=lnm, func=AF.Exp, scale=-0.5)
        rs_tiles.append(rs)

        xbf_tiles.append(xbf)
        tile_xg(t)

        # transposes: 4 groups of 8 blocks -> xt [128, 32, 128] bf16
        xt = xtpool.tile([P, 32, P], bf16, tag="xt")
        for g8 in range(4):
            pt = psum_t.tile([P, 8, P], bf16, tag="psum_t")
            for b8 in range(8):
                b = g8 * 8 + b8
                nc.tensor.transpose(
                    pt[:, b8, :], xbf[:, b * P : (b + 1) * P], ident_bf
                )
            if EVACT_ENG[g8] == "v":
                nc.vector.tensor_copy(out=xt[:, g8 * 8 : (g8 + 1) * 8, :], in_=pt)
            else:
                nc.scalar.copy(out=xt[:, g8 * 8 : (g8 + 1) * 8, :], in_=pt)
        return xt

    def tile_w(t, xt, h_pair, tl):
        wps = psum_w.tile([NK, P], f32, tag="psum_w")
        for b in range(32):
            nc.tensor.matmul(
                wps, lhsT=phiT[:, b, :], rhs=xt[:, b, :], start=(b == 0), stop=(b == 31)
            )
        wsb = w_sb[t % 2]
        nc.vector.tensor_copy(out=wsb[:NK, :], in_=wps)
        wtp = psum_wt.tile([P, P], bf16, tag="psum_wt2")
        nc.tensor.transpose(wtp, wsb[:], ident_bf)
        z = smalls.tile([P, NK], f32, tag="z")
        nc.vector.scalar_tensor_tensor(
            out=z, in0=wtp[:, :NK], scalar=rs_tiles[t], in1=beta_bc,
            op0=OP.mult, op1=OP.add,
        )
        ez = smalls.tile([P, 2 * NS], f32, tag="ez")
        nc.scalar.activation(out=ez, in_=z[:, : 2 * NS], func=AF.Exp, scale=-1.0)
        ez1 = smalls.tile([P, 2 * NS], f32, tag="ez1")
        nc.vector.tensor_scalar_add(out=ez1, in0=ez, scalar1=1.0)
        h01 = smalls.tile([P, 2 * NS], f32, tag="h01")
        nc.vector.reciprocal_approx_fast(out=h01, in_=ez1)
        h01_tiles.append(h01)
        nc.scalar.activation(
            out=h_pair[:, tl].rearrange("p j i -> p (j i)"),
            in_=z[:, 2 * NS : NK],
            func=AF.Exp,
        )
        return h_pair

    def tile_mix(t, M3bf):
        Mp = mpool.tile([P, NS, NS], bf16, tag="Mp")
        m3a = M3bf[:]
        PPi = m3a.ap[0][0]
        mp_eng = nc.sync
        for g in range(4):
            srcM = _ap(
                m3a.tensor,
                m3a.offset + g * 32 * PPi,
                [[PPi, 32], [NS, NS], [1, NS]],
            )
            mp_eng.dma_start(out=Mp[:, g, :], in_=srcM)
        lhsT = mpool.tile([P, NS, P], bf16, tag="lhsT")
        for g in range(4):
            mg = Mp[:, g, :]
            nc.gpsimd.tensor_tensor(
                out=lhsT[:, g, :].rearrange("p (i t) -> p i t", i=NS),
                in0=E,
                in1=_ap(mg.tensor, mg.offset, [mg.ap[0], [1, NS], [0, 32]]),
                op=OP.mult,
            )
        x_g = xg_tiles[t]
        out_sb = outpool.tile([P, NS, 1024], f32, tag="out_sb")
        for g in range(4):
            pm = psum_m.tile([P, 2, 512], f32, tag="psum_m")
            for hh in range(2):
                nc.tensor.matmul(
                    pm[:, hh, :],
                    lhsT=lhsT[:, g, :],
                    rhs=x_g[:, g, hh * 512 : (hh + 1) * 512],
                    start=True,
                    stop=True,
                )
            if MIX_EVAC_ENG[g] == "v":
                nc.vector.tensor_copy(out=out_sb[:, g, :], in_=pm)
            else:
                nc.scalar.copy(out=out_sb[:, g, :], in_=pm)
            # store device rows in mix order: out_dev[t*128 + (i*32+tt),
            # g*1024+c]; host unpermutes (free). Store per column-half so
            # the drain tail stays short; halves split across SP/ACT queues.
            if g % 2 == 1:
                eng = nc.sync if g == 1 else nc.scalar
                eng.dma_start(
                    out=out[t * P : (t + 1) * P, (g - 1) * 1024 : (g + 1) * 1024],
                    in_=out_sb[:, g - 1 : g + 1, :],
                )

    def tile_xg(t):
        # x_g[(tt,j), g, c] = x[g*32+tt, j*1024+c] (bf16 from xbf), emitted
        # just-in-time before the mix so the Pool stream runs casts first
        xbf = xbf_tiles[t]
        x_g = xgpool.tile([P, NS, 1024], bf16, tag="x_g")
        xba = xbf[:]
        PPx = xba.ap[0][0]
        for g in range(4):
            src = _ap(
                xba.tensor,
                xba.offset + g * 32 * PPx,
                [[PPx, 32], [1024, NS], [1, 1024]],
            )
            nc.gpsimd.dma_start(out=x_g[:, g, :], in_=src)
        xg_tiles.append(x_g)

    def tile_m3(t, h_pair, tl):
        h01 = h01_tiles[t]
        hp = h01[:, 0:NS]
        hq = h01[:, NS : 2 * NS]
        hp_b = _ap(hp.tensor, hp.offset, [hp.ap[0], hp.ap[1], [0, NS]])
        hq_b = _ap(hq.tensor, hq.offset, [hq.ap[0], [0, NS], hq.ap[1]])
        M3 = smalls.tile([P, NS, NS], f32, tag="M3")
        nc.vector.scalar_tensor_tensor(
            out=M3, in0=hp_b, scalar=2.0, in1=hq_b, op0=OP.mult, op1=OP.mult
        )
        nc.vector.tensor_add(out=M3, in0=M3, in1=h_pair[:, tl])
        M3bf = smalls.tile([P, NS, NS], bf16, tag="M3bf")
        nc.vector.tensor_copy(out=M3bf, in_=M3)
        return M3bf

    # hoist all loads: queues start moving immediately
    x_tiles = []
    for t in range(NT):
        x_t = xpool.tile([P, D], f32, tag="x_t", bufs=NT)
        load_eng[t].dma_start(out=x_t, in_=x[t * P : (t + 1) * P, :])
        x_tiles.append(x_t)

    h_pair0 = hpool.tile([P, 2, NS, NS], f32, tag="h_pair")
    h_pair1 = hpool.tile([P, 2, NS, NS], f32, tag="h_pair")
    xts = []
    for t in range(3):
        xts.append(tile_front(t, x_tiles[t]))
        tile_w(t, xts[t], h_pair0 if t < 2 else h_pair1, t % 2)
        if t == 1:
            _sinkhorn(nc, sink, h_pair0, 2, SINK_ITERS)
    tile_mix(0, tile_m3(0, h_pair0, 0))
    xts.append(tile_front(3, x_tiles[3]))
    tile_w(3, xts[3], h_pair1, 1)
    tile_mix(1, tile_m3(1, h_pair0, 1))
    _sinkhorn(nc, sink, h_pair1, 2, SINK_ITERS)
    tile_mix(2, tile_m3(2, h_pair1, 0))
    tile_mix(3, tile_m3(3, h_pair1, 1))


def build_bass(reps=1):
    nc = bacc.Bacc("TRN2", target_bir_lowering=False, debug=False)
    x = nc.dram_tensor("x", [S_SHARD, D], f32, kind="ExternalInput").ap()
    phi = nc.dram_tensor("phi", [NK, D], f32, kind="ExternalInput").ap()
    alpha = nc.dram_tensor("alpha", [3], f32, kind="ExternalInput").ap()
    beta = nc.dram_tensor("beta", [NK], f32, kind="ExternalInput").ap()
    gamma = nc.dram_tensor("gamma", [D], f32, kind="ExternalInput").ap()
    out = nc.dram_tensor("out", [S_SHARD, D], f32, kind="ExternalOutput").ap()
    with tile.TileContext(nc) as tc:
        with ExitStack() as ctx:
            _mhc_body(ctx, tc, x, phi, alpha, beta, gamma, out, reps=reps)
    nc.compile()
    return nc


_NC_CACHE = {}


def _get_nc():
    if "nc" not in _NC_CACHE:
        _NC_CACHE["nc"] = build_bass()
    return _NC_CACHE["nc"]


def make_in_maps(x, phi_weight, branch_alpha, branch_beta, norm_gamma):
    xs = np.ascontiguousarray(np.asarray(x), dtype=np.float32).reshape(S, D)
    phi = np.ascontiguousarray(np.asarray(phi_weight), dtype=np.float32)
    al = np.ascontiguousarray(np.asarray(branch_alpha), dtype=np.float32)
    be = np.ascontiguousarray(np.asarray(branch_beta), dtype=np.float32)
    ga = np.ascontiguousarray(np.asarray(norm_gamma), dtype=np.float32)
    in_maps = []
    for c in range(N_CORES):
        in_maps.append(
            {
                "x": np.ascontiguousarray(xs[c * S_SHARD : (c + 1) * S_SHARD]),
                "phi": phi,
                "alpha": al,
                "beta": be,
                "gamma": ga,
            }
        )
    return in_maps


def unpack_out(o):
    """Device rows are in mix order: out_dev[t*128 + i*32 + tt, g*1024 + c]
    holds out[t*128 + g*32 + tt, i*1024 + c]."""
    o5 = o.reshape(NT, NS, 32, NS, 1024)  # [t, i, tt, g, c]
    return np.ascontiguousarray(o5.transpose(0, 3, 2, 1, 4)).reshape(S_SHARD, D)


def kernel(x, phi_weight, branch_alpha, branch_beta, norm_gamma, _trace=False):
    nc = _get_nc()
    in_maps = make_in_maps(x, phi_weight, branch_alpha, branch_beta, norm_gamma)
    res = run_bass_kernel_spmd(
        nc, in_maps, core_ids=list(range(N_CORES)), trace=_trace
    )
    out = np.concatenate([unpack_out(r["out"]) for r in res.results], axis=0)
    if _trace:
        kernel.last_results = res
    return out.reshape(B, S, D).astype(np.float32)
